# revision 16
# baseline (speedup 1.0000x reference)
"""Trainium2 Bass kernel for nn_MoEAugmentedActor (moe_routing).

v3. Pure data parallel over 8 cores; feature-major on chip
([features(part), batch(free)], 512-sample tiles).

Structure per tile:
  - ELU(y)+1 = max(y+1, min(e^y, 1)); psum holds y+1 (bias+1 rides a
    ones contract row or a 1-contract matmul), ACT does t = Exp(psum-1),
    DVE STT finishes. The +1 shift is absorbed into the next layer's
    bias on the host.
  - Two parallel front chains: VAE (pV) and AE->gate (pA) both start
    at tile start. AE-L2 + gate-L1 fused into one [65,96] matmul.
  - Gate: gl -> tg = exp(gl+bias) on 5 rows; tg REPLICATED by selector
    matmuls (eg128 4x32 blocks; egd rows 32:61 for expert 4; denom at
    row 64), then evacuated to SBUF fp16 so the blend STTs are legal.
  - Expert L2 bias (c2+1) enters psum via a 1-contract matmul against
    the inpB ones row, so L2's ELU is the standard 2-op form with a
    shared -1 bias (1024-wide ACT per pair).
  - PSUM (8 banks): pe pool 2x[128,1024] rotating [AL1, BL1, AL2,
    E(=e4 L1|L2, then pacts1 rows 32:61), BL2]; pV 1x[128,1024]
    (VAE, then wzv|gl); pA 1x[128,512] (ae1->aeg->rep2->rep1 WAR
    chain); pacts 1x[128,512] (stacked L3 -> pbl WAR).
  - Blend is deferred into the next tile's start.
"""

import os
import sys

for _p in ("/opt/trn_rl_repo", "/root/.axon_site/_ro/trn_rl_repo"):
    if os.path.isdir(_p) and _p not in sys.path:
        sys.path.insert(0, _p)

import numpy as np

# ----------------------------------------------------------------- constants
N_FULL = 131072
N_CORES = 8
N_CORE = N_FULL // N_CORES  # 16384
TILE = 512

OBS_TERM_DIMS = (3, 3, 3, 3, 29, 29, 29, 96)
HISTORY_LEN = 5
_OFFS = [0]
for _d in OBS_TERM_DIMS[:-1]:
    _OFFS.append(_OFFS[-1] + _d * HISTORY_LEN)

VAE_COLS = [
    _OFFS[t] + i * OBS_TERM_DIMS[t] + j
    for i in range(HISTORY_LEN)
    for t in range(1, 7)
    for j in range(OBS_TERM_DIMS[t])
]  # 480
OT_COLS = [
    _OFFS[t] + 4 * OBS_TERM_DIMS[t] + j for t in range(7) for j in range(OBS_TERM_DIMS[t])
]  # 99
ELEV_COLS = list(range(_OFFS[7] + 4 * 96, _OFFS[7] + 5 * 96))  # 96

XT_ROWS = 784
WCOLS = 4224


def _w_offsets():
    off = {}
    c = 0

    def take(name, n):
        nonlocal c
        off[name] = c
        c += n

    take("w1", 4 * 256)       # 4 chunks x [<=121,256]
    take("wzv", 2 * 35)       # 2 chunks x [128,35]  ([Wv|Wz] order)
    take("ae1", 65)           # [97,65] (col 64: e_96 -> ones row in u_a)
    take("aeg", 96)           # [65,96] = [A2@G1 | A2] + bias row
    take("g2", 5)             # [64,5]
    take("rep1", 128)         # [5,128] selector (experts 0..3 -> 32e+k)
    take("rep2a", 29)         # [5,29]  expert4 selector -> rows 32:61
    take("rep2b", 1)          # [5,1]   ones (denominator) -> row 64
    take("e1a", 5 * 128)      # [128,128] x5 (o_t rows 0:99, rest zero)
    take("e1b", 5 * 128)      # [97,128] x5  (v|z 0:35, z_E 64:96, ones 96)
    take("e2", 5 * 128)       # [128,128] x5
    take("c2p1", 5 * 128)     # [1,128] x5 at prow=0 (c2_e + 1)
    take("e3", 5 * 32)        # [128,32] x5 (padded to 32)
    take("msum", 29)          # [128,29] 0/1 block-sum matrix
    take("i29", 29)           # [29,29] identity at prow=32
    assert c <= WCOLS, c
    return off


WOFF = _w_offsets()

# bpack columns
BC_ZV = 0      # rows 0..34:  [bv|bz]' adjusted
BC_G2 = 1      # rows 0..4:   gate_b2'
BC_B3 = 2      # rows 32e+k (e<4,k<29): b3'_e[k]
BC_B34 = 3     # rows 32..60: b3'_4
BC_NEG1 = 4    # all rows: -1.0
NBCOLS = 5


# ----------------------------------------------------------------- device IR

def build_program(n_rows=N_CORE, num_devices=N_CORES):
    import concourse.bass as bass
    import concourse.mybir as mybir
    from concourse import bacc
    from concourse.tile import TileContext

    fp16 = mybir.dt.float16
    fp32 = mybir.dt.float32
    AF = mybir.ActivationFunctionType
    OP = mybir.AluOpType

    n_tiles = n_rows // TILE
    assert n_rows % TILE == 0

    nc = bacc.Bacc("TRN2", target_bir_lowering=False, debug=False,
                   num_devices=num_devices)

    xT = nc.dram_tensor("xT", (XT_ROWS, n_rows), fp16, kind="ExternalInput").ap()
    wpack = nc.dram_tensor("wpack", (128, WCOLS), fp16, kind="ExternalInput").ap()
    bpack = nc.dram_tensor("bpack", (128, NBCOLS), fp32, kind="ExternalInput").ap()
    out_fm = nc.dram_tensor("out_fm", (29, n_rows), fp32, kind="ExternalOutput").ap()
    out_tg = nc.dram_tensor("out_tg", (5, n_rows), fp16, kind="ExternalOutput").ap()

    with TileContext(nc) as tc:
        with (
            tc.tile_pool(name="const", bufs=1) as constp,
            tc.tile_pool(name="xio", bufs=3) as xio,
            tc.tile_pool(name="inp", bufs=3) as inpp,
            tc.tile_pool(name="uh", bufs=2) as uhp,
            tc.tile_pool(name="small", bufs=3) as smallp,
            tc.tile_pool(name="texp", bufs=6) as texpp,
            tc.tile_pool(name="u1p", bufs=12) as u1p,
            tc.tile_pool(name="u2p", bufs=6) as u2p,
            tc.tile_pool(name="blend", bufs=3) as blendp,
            tc.tile_pool(name="px", bufs=4, space="PSUM") as pxp,
            tc.tile_pool(name="pv", bufs=1, space="PSUM") as pvp,
            tc.tile_pool(name="pa", bufs=1, space="PSUM") as pap,
            tc.tile_pool(name="pacts", bufs=1, space="PSUM") as pactsp,
        ):
            # ---- persistent constants
            wsb = constp.tile([128, WCOLS], fp16, tag="wsb")
            nc.sync.dma_start(out=wsb, in_=wpack)
            bsb = constp.tile([128, NBCOLS], fp32, tag="bsb")
            nc.sync.dma_start(out=bsb, in_=bpack)
            ones1 = constp.tile([1, TILE], fp16, tag="ones1")
            nc.vector.memset(ones1, 1.0)

            xT_blk = xT[0:640].rearrange("(b p) n -> p b n", p=128)  # [128, 5, n]

            def w(name, k, m, idx=0, msz=None, prow=0):
                base = WOFF[name] + idx * (msz if msz is not None else m)
                return wsb[prow:prow + k, base:base + m]

            def elu(pool, tag, psum, p0, p1, fd, c0=0):
                """psum[p0:p1, c0:c0+fd] holds y+1 -> u[p0:p1, 0:fd] = elu(y)+1."""
                t = texpp.tile([128, TILE], fp16, tag="texp")
                nc.scalar.activation(t[p0:p1, 0:fd], psum[p0:p1, c0:c0 + fd],
                                     AF.Exp, bias=bsb[p0:p1, BC_NEG1:BC_NEG1 + 1],
                                     scale=1.0)
                u = pool.tile([128, TILE], fp16, tag=tag)
                nc.vector.scalar_tensor_tensor(
                    out=u[p0:p1, 0:fd], in0=t[p0:p1, 0:fd], scalar=1.0,
                    in1=psum[p0:p1, c0:c0 + fd], op0=OP.min, op1=OP.max)
                return u

            def elu_wide(pool, tag, psum):
                t = texpp.tile([128, 2 * TILE], fp16, tag="texpw")
                nc.scalar.activation(t, psum, AF.Exp,
                                     bias=bsb[0:128, BC_NEG1:BC_NEG1 + 1], scale=1.0)
                u = pool.tile([128, 2 * TILE], fp16, tag=tag)
                nc.vector.scalar_tensor_tensor(
                    out=u, in0=t, scalar=1.0, in1=psum, op0=OP.min, op1=OP.max)
                return u

            # per-iteration carried state: prev = state of tile t-1
            prev = None
            for it in range(n_tiles + 1):
                n0 = it * TILE
                cur = {}
                if it < n_tiles:
                    # ---- DMAs for tile t
                    xsb = xio.tile([128, 5, TILE], fp16, tag="xsb")
                    nc.sync.dma_start(out=xsb, in_=xT_blk[:, 0:5, n0:n0 + TILE])
                    inpA = inpp.tile([128, TILE], fp16, tag="inpA")
                    nc.sync.dma_start(out=inpA, in_=xT[640:768, n0:n0 + TILE])
                    inpB = inpp.tile([97, TILE], fp16, tag="inpB")
                    nc.sync.dma_start(out=inpB[35:64], in_=xT[739:768, n0:n0 + TILE])
                    nc.sync.dma_start(out=inpB[96:97], in_=xT[771:772, n0:n0 + TILE])
                    cur["inpA"], cur["inpB"] = inpA, inpB

                    # ---- front PE: ae1 + VAE
                    pA = pap.tile([128, TILE], fp32, tag="pa")
                    nc.tensor.matmul(pA[0:65], lhsT=w("ae1", 97, 65),
                                     rhs=xsb[0:97, 4, :], start=True, stop=True)
                    pV = pvp.tile([128, 2 * TILE], fp32, tag="pv")
                    for half in (0, 1):
                        for c in range(4):
                            nc.tensor.matmul(
                                pV[:, half * TILE:(half + 1) * TILE],
                                lhsT=wsb[0:128, WOFF["w1"] + c * 256 + half * 128:
                                         WOFF["w1"] + c * 256 + half * 128 + 128],
                                rhs=xsb[0:128, c, :],
                                start=(c == 0), stop=(c == 3))
                    u_a = elu(uhp, "ua", pA, 0, 65, TILE)
                    u_h = elu_wide(uhp, "uh", pV)
                    nc.tensor.matmul(pA[0:96], lhsT=w("aeg", 65, 96),
                                     rhs=u_a[0:65, 0:TILE], start=True, stop=True)
                    nc.scalar.activation(inpB[64:96], pA[64:96], AF.Identity,
                                         bias=0.0, scale=1.0)
                    u_g = elu(uhp, "ug", pA, 0, 64, TILE)
                    cur["pA"] = pA

                # ---- Y(t-1): expert L2 lanes 0,1
                if prev is not None:
                    p = prev
                    p["pl2"] = []
                    p["u2"] = []

                    def l2mm(e):
                        pl2 = pxp.tile([128, TILE], fp32, tag="px")
                        nc.tensor.matmul(pl2, lhsT=w("e2", 128, 128, e),
                                         rhs=p["u1"][e], start=True, stop=False)
                        nc.tensor.matmul(pl2, lhsT=w("c2p1", 1, 128, e),
                                         rhs=ones1, start=False, stop=True)
                        p["pl2"].append(pl2)

                    l2mm(0)
                    l2mm(1)
                    p["u2"].append(elu(u2p, "u2", p["pl2"][0], 0, 128, TILE))

                if it < n_tiles:
                    # ---- wzv + gl (second pV alloc)
                    pS = pvp.tile([128, 2 * TILE], fp32, tag="pv")
                    nc.tensor.matmul(pS[0:35, 0:TILE],
                                     lhsT=w("wzv", 128, 35, 0, msz=35),
                                     rhs=u_h[:, 0:TILE], start=True, stop=False)
                    nc.tensor.matmul(pS[0:35, 0:TILE],
                                     lhsT=w("wzv", 128, 35, 1, msz=35),
                                     rhs=u_h[:, TILE:2 * TILE], start=False, stop=True)
                    nc.scalar.activation(inpB[0:35], pS[0:35, 0:TILE], AF.Identity,
                                         bias=bsb[0:35, BC_ZV:BC_ZV + 1], scale=1.0)

                if prev is not None:
                    l2mm(2)
                    prev["u2"].append(elu(u2p, "u2", prev["pl2"][1], 0, 128, TILE))

                if it < n_tiles:
                    nc.tensor.matmul(pS[0:5, TILE:2 * TILE], lhsT=w("g2", 64, 5),
                                     rhs=u_g[0:64, 0:TILE], start=True, stop=True)
                    tg = smallp.tile([5, TILE], fp16, tag="tg")
                    nc.scalar.activation(tg, pS[0:5, TILE:2 * TILE], AF.Exp,
                                         bias=bsb[0:5, BC_G2:BC_G2 + 1], scale=1.0)

                if prev is not None:
                    l2mm(3)
                    prev["u2"].append(elu(u2p, "u2", prev["pl2"][2], 0, 128, TILE))
                    # L3 for lanes 0,1 into pacts0(t-1)
                    pacts0 = pactsp.tile([128, TILE], fp32, tag="pacts")
                    prev["pacts0"] = pacts0

                    def l3mm(e):
                        nc.tensor.matmul(pacts0[32 * e:32 * e + 32],
                                         lhsT=w("e3", 128, 32, e, msz=32),
                                         rhs=prev["u2"][e], start=True, stop=True,
                                         tile_position=(0, 32 * e))

                    l3mm(0)

                if it < n_tiles:
                    # ---- gate replication (WAR over pA) + evacuations
                    nc.tensor.matmul(pA[32:61], lhsT=w("rep2a", 5, 29), rhs=tg,
                                     start=True, stop=True)
                    nc.sync.dma_start(out=out_tg[:, n0:n0 + TILE], in_=tg)
                    egd16 = blendp.tile([61, TILE], fp16, tag="egd16")
                    nc.scalar.activation(egd16[32:61], pA[32:61], AF.Identity,
                                         bias=0.0, scale=1.0)
                    cur["egd16"] = egd16

                if prev is not None:
                    l2mm(4)
                    prev["u2"].append(elu(u2p, "u2", prev["pl2"][3], 0, 128, TILE))
                    l3mm(1)

                if it < n_tiles:
                    nc.tensor.matmul(pA[0:128], lhsT=w("rep1", 5, 128), rhs=tg,
                                     start=True, stop=True)
                    eg16 = blendp.tile([128, TILE], fp16, tag="eg16")
                    nc.scalar.activation(eg16, pA[0:128], AF.Identity,
                                         bias=0.0, scale=1.0)
                    cur["eg16"] = eg16

                if prev is not None:
                    prev["u2"].append(elu(u2p, "u2", prev["pl2"][4], 0, 128, TILE))
                    l3mm(2)
                    l3mm(3)
                    # e4 L3 -> pacts1 in a px alloc (rows 32:61)
                    pacts1 = pxp.tile([128, TILE], fp32, tag="px")
                    nc.tensor.matmul(pacts1[32:61], lhsT=w("e3", 128, 29, 4, msz=32),
                                     rhs=prev["u2"][4], start=True, stop=True,
                                     tile_position=(0, 32))

                    # ---- blend(t-1)
                    p = prev
                    s_all = blendp.tile([128, TILE], fp16, tag="s_all")
                    nc.vector.scalar_tensor_tensor(
                        out=s_all, in0=pacts0, scalar=bsb[0:128, BC_B3:BC_B3 + 1],
                        in1=p["eg16"], op0=OP.add, op1=OP.mult)
                    se4 = blendp.tile([61, TILE], fp16, tag="se4")
                    nc.vector.scalar_tensor_tensor(
                        out=se4[32:61], in0=pacts1[32:61],
                        scalar=bsb[32:61, BC_B34:BC_B34 + 1],
                        in1=p["egd16"][32:61], op0=OP.add, op1=OP.mult)
                    nc.tensor.matmul(pacts0[0:29], lhsT=w("msum", 128, 29),
                                     rhs=s_all, start=True, stop=False)
                    nc.tensor.matmul(pacts0[0:29], lhsT=w("i29", 29, 29, prow=32),
                                     rhs=se4[32:61], start=False, stop=True)
                    acc = blendp.tile([29, TILE], fp32, tag="acc")
                    nc.scalar.activation(acc, pacts0[0:29], AF.Identity,
                                         bias=0.0, scale=1.0)
                    nc.sync.dma_start(out=out_fm[:, (it - 1) * TILE:it * TILE],
                                      in_=acc)

                if it < n_tiles:
                    # ---- expert L1 lanes for tile t (inpB now ready)
                    cur["u1"] = []
                    for e in range(5):
                        pl1 = pxp.tile([128, TILE], fp32, tag="px")
                        nc.tensor.matmul(pl1, lhsT=w("e1a", 128, 128, e),
                                         rhs=inpA, start=True, stop=False)
                        nc.tensor.matmul(pl1, lhsT=w("e1b", 97, 128, e),
                                         rhs=inpB, start=False, stop=True)
                        cur["u1"].append(elu(u1p, "u1", pl1, 0, 128, TILE))

                prev = cur if it < n_tiles else None
    nc.compile()
    return nc


# ----------------------------------------------------------------- host prep

def prep_inputs(x, vae_W1, vae_b1, vae_Wz, vae_bz, vae_Wv, vae_bv,
                ae_W1, ae_b1, ae_W2, ae_b2,
                gate_W1, gate_b1, gate_W2, gate_b2,
                eW1, eb1, eW2, eb2, eW3, eb3, n_rows=N_CORE, n_cores=N_CORES):
    x = np.asarray(x, np.float32)
    n_total = n_rows * n_cores
    assert x.shape[0] >= n_total

    xT = np.zeros((XT_ROWS, n_total), np.float16)
    xv = x[:n_total, VAE_COLS].T.astype(np.float16)  # [480, n]
    for c in range(4):
        xT[128 * c:128 * c + 120] = xv[120 * c:120 * c + 120]
    xT[504] = 1.0
    xT[512:608] = x[:n_total, ELEV_COLS].T.astype(np.float16)
    xT[608] = 1.0
    xT[640:739] = x[:n_total, OT_COLS].T.astype(np.float16)
    xT[771] = 1.0

    wpack = np.zeros((128, WCOLS), np.float32)
    bpack = np.zeros((128, NBCOLS), np.float32)
    bpack[:, BC_NEG1] = -1.0

    def put(name, idx, arr, msz=None, prow=0):
        k, m = arr.shape
        base = WOFF[name] + idx * (msz if msz is not None else m)
        wpack[prow:prow + k, base:base + m] = arr

    W1 = np.asarray(vae_W1, np.float32)  # [480, 256]
    for c in range(4):
        chunk = W1[120 * c:120 * c + 120]
        if c == 3:
            chunk = np.vstack([chunk, (np.asarray(vae_b1) + 1.0)[None]])
        put("w1", c, chunk, msz=256)
    Wzv = np.concatenate([vae_Wv, vae_Wz], axis=1).astype(np.float32)  # [256,35]
    put("wzv", 0, Wzv[0:128], msz=35)
    put("wzv", 1, Wzv[128:256], msz=35)
    bpack[0:35, BC_ZV] = np.concatenate([vae_bv, vae_bz]) - Wzv.sum(0)

    # ae1: [97,65]; cols 0:64 = [ae_W1; ae_b1+1]; col 64 = e_96 (ones row)
    ae1 = np.zeros((97, 65), np.float32)
    ae1[0:96, 0:64] = np.asarray(ae_W1, np.float32)
    ae1[96, 0:64] = np.asarray(ae_b1) + 1.0
    ae1[96, 64] = 1.0
    put("ae1", 0, ae1)

    # aeg: [65,96] from u_a (= [ha+1; 1]): gate hidden cols 0:64, z_E 64:96
    A2 = np.asarray(ae_W2, np.float32)       # [64,32]
    G1 = np.asarray(gate_W1, np.float32)     # [32,64]
    bz = np.asarray(ae_b2, np.float32) - A2.sum(0)          # [32]
    aeg = np.zeros((65, 96), np.float32)
    aeg[0:64, 0:64] = A2 @ G1
    aeg[64, 0:64] = bz @ G1 + np.asarray(gate_b1) + 1.0
    aeg[0:64, 64:96] = A2
    aeg[64, 64:96] = bz
    put("aeg", 0, aeg)

    G2 = np.asarray(gate_W2, np.float32)  # [64,5]
    put("g2", 0, G2)
    bpack[0:5, BC_G2] = np.asarray(gate_b2) - G2.sum(0)

    rep1 = np.zeros((5, 128), np.float32)
    for e in range(4):
        rep1[e, 32 * e:32 * e + 29] = 1.0
    put("rep1", 0, rep1)
    rep2a = np.zeros((5, 29), np.float32)
    rep2a[4, :] = 1.0
    put("rep2a", 0, rep2a)

    for e in range(5):
        W1e = np.asarray(eW1[e], np.float32)  # [166,128]
        put("e1a", e, W1e[0:99], msz=128)
        e1b = np.zeros((97, 128), np.float32)
        e1b[0:35] = W1e[99:134]      # v_pred(3) + z_H(32)
        e1b[64:96] = W1e[134:166]    # z_E
        e1b[96] = np.asarray(eb1[e]) + 1.0
        put("e1b", e, e1b, msz=128)
        W2e = np.asarray(eW2[e], np.float32)
        c2 = np.asarray(eb2[e]) - W2e.sum(0)
        put("e2", e, W2e, msz=128)
        put("c2p1", e, (c2 + 1.0)[None, :], msz=128)
        W3e = np.asarray(eW3[e], np.float32)
        W3p = np.zeros((128, 32), np.float32)
        W3p[:, 0:29] = W3e
        put("e3", e, W3p, msz=32)
        b3e = np.asarray(eb3[e]) - W3e.sum(0)  # [29]
        if e < 4:
            bpack[32 * e:32 * e + 29, BC_B3] = b3e
        else:
            bpack[32:61, BC_B34] = b3e
    msum = np.zeros((128, 29), np.float32)
    for e in range(4):
        msum[32 * e:32 * e + 29] = np.eye(29)
    put("msum", 0, msum)
    put("i29", 0, np.eye(29, dtype=np.float32), prow=32)

    wpack16 = wpack.astype(np.float16)
    in_maps = []
    for c in range(n_cores):
        in_maps.append({
            "xT": np.ascontiguousarray(xT[:, c * n_rows:(c + 1) * n_rows]),
            "wpack": wpack16,
            "bpack": bpack,
        })
    return in_maps


# ----------------------------------------------------------------- entry

_NC_CACHE = {}


def _get_program(n_rows=N_CORE, num_devices=N_CORES):
    key = (n_rows, num_devices)
    if key not in _NC_CACHE:
        _NC_CACHE[key] = build_program(n_rows, num_devices)
    return _NC_CACHE[key]


def kernel(**inputs):
    from concourse.bass_utils import run_bass_kernel_spmd

    nc = _get_program()
    in_maps = prep_inputs(**inputs)
    res = run_bass_kernel_spmd(nc, in_maps, core_ids=list(range(N_CORES)))
    out = np.empty((N_FULL, 29), np.float32)
    for c in range(N_CORES):
        pbl = res.results[c]["out_fm"]                      # [29, n] unnormalized
        tg = res.results[c]["out_tg"].astype(np.float32)    # [5, n]
        out[c * N_CORE:(c + 1) * N_CORE] = (pbl / tg.sum(0)[None, :]).T
    return out
